# revision 16
# baseline (speedup 1.0000x reference)
"""Multi-head self-attention Trainium2 kernel (B=2, S=2048, D=1024, H=32, d=32).

Sharding: 8 cores = (batch b in {0,1}) x (query quarter qc in {0..3}).
Each core holds x[b] fully (keys) and computes attention + output projection
for its 512 queries. Host concatenates + adds bo.

Per-core pipeline (bf16 operands, fp32 PSUM):
  - Q/K projections folded: Gt = wq @ wk.T on host; yq = blockdiag4(Gt).T @ xqT
    computed once per head-group up front, so no K-projection and no kt
    evacuation at all. scores = xT . yq with K=32 row-tiling (4 heads
    concurrent on the PE).
  - V-projection folded: attn@v contracts exp-scores against RAW x augmented
    with a ones column (per head 33 lhsT cols) -> U (= x^T e) and the softmax
    denominator accumulate together in PSUM across all 16 key chunks; wv is
    applied afterwards to the tiny [128, 512] U per head-group (2 col-tiled
    matmuls), so no V-projection over the 2048 keys either.
  - exp split across two engines: ACT does exact exp (with fused 1/sqrt(d)
    scale) on the first EXP_ACT columns of each [128,1024] score tile; the DVE
    does a one-instruction Schraudolph exp on the rest: out_int16 =
    round(s * 128*SCALE/ln2 + 128*(127-C)), bit-viewed as bf16. Both write
    disjoint column ranges of the same bf16 et tile.
  - attn@v: M=33 (32 v-dims + ones), 2-way column tiling at positions
    (0,0)/(0,64); two head-pairs per (hg,kc) into po_A/po_B psum banks.
  - normalization: denominator rows broadcast via 4 col-tiled K=1 matmuls,
    reciprocal_approx_fast, one tensor_mul into the CT concat tile.
  - out-projection: CT chunks as lhsT against natural-order wo (bf16 rhs,
    N=1024), accumulated fully in PSUM (start/stop over 8 chunks), evacuated
    once to bf16 and DMA'd out.
PSUM budget: scores pool 3x[128,1024]f32 (6 banks) + po 2x[128,512] (2 banks);
hg-tail transients and out-proj tiles reuse the scores pool slots.
"""
import numpy as np
import ml_dtypes

import concourse.bacc as bacc
import concourse.mybir as mybir
import concourse.tile as tile
from concourse import bass_utils

f32 = mybir.dt.float32
bf16 = mybir.dt.bfloat16
i16 = mybir.dt.int16
AF = mybir.ActivationFunctionType
ALU = mybir.AluOpType

B, S, D, H, dh = 2, 2048, 1024, 32, 32
NCORES = 8
QCH = S // (NCORES // B)      # 512 queries per core
NHG = D // 128                # 8 four-head groups
NKC = S // 128                # 16 key chunks
SCALE = 1.0 / float(np.sqrt(dh))

# Schraudolph bf16 exp constants (round-to-nearest-even int16 convert)
SCH_C = 0.0435
SCH_A = 128.0 * SCALE / float(np.log(2.0))
SCH_B = 128.0 * (127.0 - SCH_C)
# per-[128,1024]-tile exp engine cost (ns): ACT (172+1024)/1.2, DVE
# (120+1024)/0.96 -- whole-tile assignment pays instruction overhead once
ACT_TILE_NS = 997.0
DVE_TILE_NS = 1192.0
ACT_AUX_NS = 18000.0   # yq/usb/osb evacuations
DVE_AUX_NS = 12000.0   # recip + CT-mul


def build_module(loop_iters: int = 0):
    nc = bacc.Bacc("TRN2", target_bir_lowering=False, debug=False)
    xt_d = nc.dram_tensor("xt", [D, S], bf16, kind="ExternalInput")
    xtq_d = nc.dram_tensor("xtq", [D, QCH], bf16, kind="ExternalInput")
    xa_d = nc.dram_tensor("xa", [S, NHG * 132], bf16, kind="ExternalInput")
    gt4_d = nc.dram_tensor("gt4", [128, 128], bf16, kind="ExternalInput")
    wvab_d = nc.dram_tensor("wvab", [128, 128], bf16, kind="ExternalInput")
    wo_d = nc.dram_tensor("wo8", [D, D], bf16, kind="ExternalInput")
    out_d = nc.dram_tensor("out", [QCH, D], bf16, kind="ExternalOutput")

    with tile.TileContext(nc) as tc:
        with (
            tc.tile_pool(name="pers", bufs=1) as pers,
            tc.tile_pool(name="sbyq", bufs=1) as sbyq,
            tc.tile_pool(name="sbe", bufs=4) as sbe,
            tc.tile_pool(name="sbu", bufs=2) as sbu,
            tc.tile_pool(name="sbr", bufs=2) as sbr,
            tc.tile_pool(name="sbo", bufs=4) as sbo,
            tc.tile_pool(name="psS", bufs=3, space="PSUM") as psS,
            tc.tile_pool(name="psO", bufs=1, space="PSUM") as psO,
        ):
            def body(_iv=None):
                # load order matters: XTQ+gt4 feed the upfront yq phase, XT0
                # and the XA tiles feed hg0's scores/attnv, XT1-7 arrive one
                # hg ahead of use. All on the otherwise-idle SP queue.
                gt4 = pers.tile([128, 128], bf16, name="gt4", tag="gt4")
                nc.sync.dma_start(gt4[:, :], gt4_d[:, :])
                wvab = pers.tile([128, 128], bf16, name="wvab", tag="wvab")
                nc.sync.dma_start(wvab[:, :], wvab_d[:, :])
                XTQ = []
                for t in range(NHG):
                    xqt = pers.tile([128, QCH], bf16, name=f"XTQ{t}",
                                    tag=f"XTQ{t}")
                    XTQ.append(xqt)
                XT = []
                for t in range(NHG):
                    xtt = pers.tile([128, S], bf16, name=f"XT{t}", tag=f"XT{t}")
                    XT.append(xtt)
                nc.gpsimd.dma_start(XTQ[0][:, :], xtq_d[0:128, :])
                nc.gpsimd.dma_start(XT[0][:, :], xt_d[0:128, :])
                for t in range(1, NHG):
                    nc.gpsimd.dma_start(XTQ[t][:, :],
                                        xtq_d[128 * t:128 * (t + 1), :])
                XA = []
                for t in range(NKC):
                    xat = pers.tile([128, NHG * 132], bf16, name=f"XA{t}",
                                    tag=f"XA{t}")
                    nc.gpsimd.dma_start(xat[:, :],
                                        xa_d[128 * t:128 * (t + 1), :])
                    XA.append(xat)
                for t in range(1, NHG):
                    nc.gpsimd.dma_start(XT[t][:, :],
                                        xt_d[128 * t:128 * (t + 1), :])
                WO = []
                for t in range(NHG):
                    wot = pers.tile([128, D], bf16, name=f"WO{t}", tag=f"WO{t}")
                    nc.sync.dma_start(wot[:, :], wo_d[128 * t:128 * (t + 1), :])
                    WO.append(wot)
                onesb = pers.tile([128, 64], bf16, name="onesb", tag="onesb")
                nc.vector.memset(onesb[:, :], 1.0)
                zrow = pers.tile([1, 640], bf16, name="zrow", tag="zrow")
                nc.vector.memset(zrow[:, :], 0.0)

                # ---- all 8 yq projections up front ----
                YQ = sbyq.tile([128, NHG * QCH], bf16, name="YQ", tag="YQ")
                for hg in range(NHG):
                    pq = psS.tile([128, 1024], f32, name=f"pq{hg}", tag="ss")
                    nc.tensor.matmul(pq[:, 0:QCH], gt4[:, :], XTQ[hg][:, :],
                                     start=True, stop=True)
                    nc.scalar.activation(YQ[:, QCH * hg:QCH * (hg + 1)],
                                         pq[:, 0:QCH], AF.Copy)

                CT = sbyq.tile([128, NHG * QCH], bf16, name="CT", tag="CT")

                def make_tail(hg, usb):
                    # denominator broadcast + recip + wv-apply + normalize;
                    # deferred one hg so its PE/DVE work never stalls the
                    # next group's score/exp stream
                    def tail():
                        dbc = psS.tile([128, 1024], f32, name=f"dbc{hg}",
                                       tag="ss")
                        for j, (dr, co) in enumerate(
                                ((32, 0), (96, 0), (32, QCH), (96, QCH))):
                            nc.tensor.matmul(
                                dbc[32 * j:32 * (j + 1), 0:QCH],
                                onesb[dr:dr + 1, 0:32],
                                usb[dr:dr + 1, co:co + QCH],
                                start=True, stop=True,
                                tile_position=(dr, 32 * j),
                                skip_group_check=True)
                        rn = sbr.tile([128, QCH], f32, name=f"rn{hg}",
                                      tag="rn")
                        nc.vector.reciprocal_approx_fast(rn[:, :],
                                                         dbc[:, 0:QCH])
                        ctp = psS.tile([128, 1024], f32, name=f"ctp{hg}",
                                       tag="ss")
                        for r, co in ((0, 0), (64, QCH)):
                            nc.tensor.matmul(
                                ctp[r:r + 64, 0:QCH],
                                wvab[:, r:r + 64],
                                usb[:, co:co + QCH],
                                start=True, stop=True,
                                tile_position=(0, r),
                                skip_group_check=True)
                        nc.vector.tensor_mul(CT[:, QCH * hg:QCH * (hg + 1)],
                                             ctp[:, 0:QCH], rn[:, :])
                    return tail

                pending_tail = None
                eng_t = [ACT_AUX_NS, DVE_AUX_NS]   # running ACT/DVE load (ns)
                for hg in range(NHG):
                    # openers: zero matmuls set has_written over both po banks
                    po = psO.tile([128, 1024], f32, name=f"po{hg}", tag="po")
                    for bank in range(2):
                        nc.tensor.matmul(po[:, QCH * bank:QCH * (bank + 1)],
                                         zrow[:1, 0:128], zrow[:1, 128:640],
                                         start=True, stop=True,
                                         skip_group_check=True)

                    prev = None

                    def attnv(kc, ets):
                        for j in range(4):
                            co, tp = QCH * (j // 2), 64 * (j % 2)
                            nc.tensor.matmul(
                                po[tp:tp + 33, co:co + QCH],
                                XA[kc][:, 132 * hg + 33 * j:
                                       132 * hg + 33 * (j + 1)],
                                ets[j // 2][:, QCH * (j % 2):
                                            QCH * (j % 2 + 1)],
                                start=False, stop=(kc == NKC - 1),
                                tile_position=(0, tp),
                                skip_group_check=True)

                    for kc in range(NKC):
                        ets = []
                        for pr in range(2):
                            ss = psS.tile([128, 1024], f32,
                                          name=f"ss{hg}_{kc}_{pr}", tag="ss")
                            for jj in range(2):
                                j = 2 * pr + jj
                                nc.tensor.matmul(
                                    ss[:, QCH * jj:QCH * (jj + 1)],
                                    XT[hg][32 * j:32 * (j + 1),
                                           128 * kc:128 * (kc + 1)],
                                    YQ[32 * j:32 * (j + 1),
                                       QCH * hg:QCH * (hg + 1)],
                                    start=True, stop=True,
                                    tile_position=(32 * j, 0),
                                    skip_group_check=True)
                            et = sbe.tile([128, 1024], bf16,
                                          name=f"et{hg}_{kc}_{pr}", tag="et")
                            # whole-tile engine assignment, greedy-balanced
                            if eng_t[0] + ACT_TILE_NS <= eng_t[1] + DVE_TILE_NS:
                                eng_t[0] += ACT_TILE_NS
                                nc.scalar.activation(et[:, :], ss[:, :],
                                                     AF.Exp, scale=SCALE)
                            else:
                                eng_t[1] += DVE_TILE_NS
                                nc.vector.tensor_scalar(
                                    et[:, :].bitcast(i16), ss[:, :],
                                    SCH_A, SCH_B, ALU.mult, ALU.add)
                            ets.append(et)
                        if prev is not None:
                            attnv(*prev)
                        prev = (kc, ets)
                    attnv(*prev)

                    # ---- evacuate U+denominators now (frees po for next hg);
                    # the rest of the tail is deferred one hg
                    usb = sbu.tile([128, 1024], bf16, name=f"usb{hg}",
                                   tag="usb")
                    nc.scalar.activation(usb[:, :], po[:, :], AF.Copy)
                    if pending_tail is not None:
                        pending_tail()
                    pending_tail = make_tail(hg, usb)

                if pending_tail is not None:
                    pending_tail()

                # ---- output projection, PSUM-accumulated ----
                for qs in range(QCH // 128):
                    op = psS.tile([128, 1024], f32, name=f"op{qs}", tag="ss")
                    for c in range(NHG):
                        for og in range(2):
                            nc.tensor.matmul(
                                op[:, 512 * og:512 * (og + 1)],
                                CT[:, QCH * c + 128 * qs:
                                   QCH * c + 128 * (qs + 1)],
                                WO[c][:, 512 * og:512 * (og + 1)],
                                start=(c == 0), stop=(c == NHG - 1),
                                skip_group_check=True)
                    osb = sbo.tile([128, D], bf16, name=f"osb{qs}",
                                   tag=f"osb{qs}")
                    nc.scalar.activation(osb[:, :], op[:, :], AF.Copy)
                    nc.sync.dma_start(out_d[128 * qs:128 * (qs + 1), :],
                                      osb[:, :])

            if loop_iters > 0:
                with tc.For_i(0, loop_iters, 1):
                    body()
            else:
                body()

    nc.compile()
    return nc


def _prep_inputs(x, wq, bq, wk, bk, wv, bv, wo, bo):
    x = np.asarray(x, dtype=np.float32)
    wq = np.asarray(wq, dtype=np.float32)
    wk = np.asarray(wk, dtype=np.float32)
    wv = np.asarray(wv, dtype=np.float32)
    wo = np.asarray(wo, dtype=np.float32)
    for name, b_ in (("bq", bq), ("bk", bk), ("bv", bv)):
        if np.any(np.asarray(b_) != 0):
            raise NotImplementedError(f"nonzero {name} not supported")
    bfl = ml_dtypes.bfloat16

    def blockdiag(w):
        o = np.zeros((128, 128), np.float32)
        for i in range(4):
            o[32 * i:32 * (i + 1), 32 * i:32 * (i + 1)] = w
        return o

    # gt4: lhsT for yq projection; yq = gt4.T @ xqT per 4-head group
    gt4 = blockdiag(wq.astype(np.float64) @ wk.astype(np.float64).T)

    # wv-apply lhsT [128, 128]: cols 0-63 = region A (heads j=0 from rows
    # 0-31 -> cols 0-31, j=1 from rows 64-95 -> cols 32-63); cols 64-127 =
    # region B same pattern (heads j=2, j=3). d-rows (32/96) stay zero.
    wvab = np.zeros((128, 128), np.float32)
    for half in range(2):
        co = 64 * half
        wvab[0:32, co + 0:co + 32] = wv
        wvab[64:96, co + 32:co + 64] = wv

    # xa: [S, NHG*132]: per hg block of 132 = 4 heads x (32 x-cols + ones)
    def make_xa(xb):
        xa = np.ones((S, NHG * 132), np.float32)
        for hg in range(NHG):
            for j in range(4):
                base = 132 * hg + 33 * j
                xa[:, base:base + 32] = xb[:, 128 * hg + 32 * j:
                                           128 * hg + 32 * (j + 1)]
        return xa

    shared = {
        "gt4": gt4.astype(bfl),
        "wvab": wvab.astype(bfl),
        "wo8": wo.astype(bfl),
    }
    in_maps = []
    for c in range(NCORES):
        b, qc = c // (NCORES // B), c % (NCORES // B)
        xb = x[b]
        xt = np.ascontiguousarray(xb.T).astype(bfl)
        m = dict(shared)
        m["xt"] = xt
        m["xtq"] = np.ascontiguousarray(
            xt[:, QCH * qc:QCH * (qc + 1)])
        m["xa"] = make_xa(xb).astype(bfl)
        in_maps.append(m)
    return in_maps


_NC_CACHE = {}


def kernel(x, wq, bq, wk, bk, wv, bv, wo, bo):
    in_maps = _prep_inputs(x, wq, bq, wk, bk, wv, bv, wo, bo)
    if "nc" not in _NC_CACHE:
        _NC_CACHE["nc"] = build_module()
    nc = _NC_CACHE["nc"]
    res = bass_utils.run_bass_kernel_spmd(nc, in_maps,
                                          core_ids=list(range(NCORES)))
    out = np.empty((B, S, D), np.float32)
    for c in range(NCORES):
        b, qc = c // (NCORES // B), c % (NCORES // B)
        out[b, QCH * qc:QCH * (qc + 1), :] = res.results[c]["out"].astype(
            np.float32)
    out += np.asarray(bo, dtype=np.float32)[None, None, :]
    return out
